# revision 8
# baseline (speedup 1.0000x reference)
"""Trainium2 Bass kernel for sparse CausalSelfAttention (8 full heads W=1024,
8 reduced-qk heads W=256), SPMD over 8 NeuronCores.

Sharding: core c -> batch c//4, head-group g=c%4 (full heads 2g,2g+1 and
reduced heads 2g,2g+1). Each core computes its QKV projection slices, windowed
attention in transposed layout, and a partial c_proj against its 256-row slice
of w_proj. Host sums the 4 partials per batch element.
"""

import numpy as np

import concourse.bacc as bacc
import concourse.mybir as mybir
from concourse import bass_utils
from concourse.tile import TileContext

# problem constants (hardcoded; kernel.py must be self-contained)
B, T, C = 2, 2048, 1024
HDIM = 64          # full head dim (and v dim of reduced heads)
RDIM = 32          # reduced qk dim
WF, WR = 1024, 256  # windows
QF, QR = 512, 256   # query-block sizes
N_CORES = 8
NK = C // 128       # k-tiles over C contraction

F32R = mybir.dt.float32r
F32 = mybir.dt.float32

# full-head mask offsets d = i0 - j0 (Q=512, W=1024): 1.0 where 0 <= d+f-p < W
MASKF_D = [0, -128, -256, -384, 640, 768, 896, 1024]
# reduced-head mask offsets (Q=256, W=256)
MASKR_D = [256, 128, 0, -128]
CLEAN_F = {128, 256, 384, 512}  # fully-valid offsets for full heads


def _full_mask_idx(d):
    if d in CLEAN_F:
        return None
    if d <= 0:
        return -d // 128
    return 4 + (d - 640) // 128


def _make_mask(nc, dst, d, w):
    """dst[p, f] = 1.0 where 0 <= d + f - p < w else 0.0 (on gpsimd)."""
    q = dst.shape[-1]
    nc.gpsimd.memset(dst, 1.0)
    # keep where (d + f - p) >= 0
    nc.gpsimd.affine_select(out=dst, in_=dst, compare_op=mybir.AluOpType.is_ge,
                            fill=0.0, base=d, pattern=[[1, q]],
                            channel_multiplier=-1)
    # keep where (w - 1 - d + p - f) >= 0  i.e. d + f - p < w
    nc.gpsimd.affine_select(out=dst, in_=dst, compare_op=mybir.AluOpType.is_ge,
                            fill=0.0, base=w - 1 - d, pattern=[[-1, q]],
                            channel_multiplier=1)


def _emit_body(nc, pools, aps):
    wpool, xpool, qkpool, ppool, opool, rpool, ps_misc, ps_s, ps_y = pools
    xT, wq, wk, wqr, wkr, wv, wproj, out = aps

    # ---- weight loads (in first-use order) ----
    wq_sb = wpool.tile([128, NK, 128], F32R, tag="wq")
    wk_sb = wpool.tile([128, NK, 128], F32R, tag="wk")
    wqr_sb = wpool.tile([128, NK, 128], F32R, tag="wqr")
    wkr_sb = wpool.tile([128, NK, 128], F32R, tag="wkr")
    wv_sb = wpool.tile([128, NK, 256], F32R, tag="wv")
    wproj_sb = wpool.tile([128, 2, C], F32R, tag="wproj")
    for k in range(NK):
        nc.sync.dma_start(wq_sb[:, k, :], wq[k * 128:(k + 1) * 128, :])
    for k in range(NK):
        nc.sync.dma_start(wk_sb[:, k, :], wk[k * 128:(k + 1) * 128, :])
    for k in range(NK):
        nc.sync.dma_start(wqr_sb[:, k, :], wqr[k * 128:(k + 1) * 128, :])
    for k in range(NK):
        nc.sync.dma_start(wkr_sb[:, k, :], wkr[k * 128:(k + 1) * 128, :])
    for k in range(NK):
        nc.sync.dma_start(wv_sb[:, k, :], wv[k * 128:(k + 1) * 128, :])

    # ---- masks generated on gpsimd (keeps the DMA queue for x/weights) ----
    mf_sb = wpool.tile([128, len(MASKF_D), QF], F32R, tag="mf")
    mr_sb = wpool.tile([128, len(MASKR_D), QR], F32R, tag="mr")
    for m, d in enumerate(MASKF_D):
        _make_mask(nc, mf_sb[:, m, :].bitcast(F32), d, WF)
    for m, d in enumerate(MASKR_D):
        _make_mask(nc, mr_sb[:, m, :].bitcast(F32), d, WR)

    # persistent transposed activations [dim-stack, T]
    qTf = qkpool.tile([128, T], F32R, tag="qTf")  # rows: hA q (64) | hB q (64)
    kTf = qkpool.tile([128, T], F32R, tag="kTf")
    qTr = qkpool.tile([128, T], F32R, tag="qTr")  # rows: rA qr|0|rB qr|0
    kTr = qkpool.tile([128, T], F32R, tag="kTr")
    # v values + ones block: [128, T-tile, head, 128] (cols 64:128 = 1.0)
    v_sb = qkpool.tile([128, T // 128, 4, 128], F32R, tag="v")
    nc.gpsimd.memset(v_sb[:, :, :, 64:128].bitcast(F32), 1.0)
    # attention outputs yT (normalized), stacked per pair
    yTf = qkpool.tile([128, T], F32R, tag="yTf")
    yTr = qkpool.tile([128, T], F32R, tag="yTr")

    # ---- phase B: projections, streaming xT by T-block of 512 ----
    for tb in range(T // 512):
        sl = slice(tb * 512, (tb + 1) * 512)
        xts = []
        for k in range(NK):
            xt = xpool.tile([128, 512], F32R, tag="xt")
            nc.sync.dma_start(xt[:], xT[k * 128:(k + 1) * 128, sl])
            xts.append(xt)
        for w_sb, dst in ((wq_sb, qTf), (wk_sb, kTf),
                          (wqr_sb, qTr), (wkr_sb, kTr)):
            psum = ps_misc.tile([128, 512], F32, tag="m")
            for k in range(NK):
                nc.tensor.matmul(psum[:], w_sb[:, k, :], xts[k][:],
                                 start=(k == 0), stop=(k == NK - 1))
            nc.vector.tensor_copy(dst[:, sl], psum[:])
        for tt in range(4):
            gt = tb * 4 + tt  # global T-tile
            psv = ps_misc.tile([128, 256], F32, tag="m")
            for k in range(NK):
                nc.tensor.matmul(psv[:], xts[k][:, tt * 128:(tt + 1) * 128],
                                 wv_sb[:, k, :],
                                 start=(k == 0), stop=(k == NK - 1))
            nc.vector.tensor_copy(
                v_sb[:, gt, :, 0:64],
                psv[:].rearrange("p (h d) -> p h d", h=4))

    # w_proj load late: first needed by phase D interleaved into attention
    for k in range(2):
        nc.sync.dma_start(wproj_sb[:, k, :], wproj[k * 128:(k + 1) * 128, :])

    # ---- phase C + D interleaved ----
    def attn_block(qT, kT_, Q, W, m_sb, mask_d, heads, yT, is_full, qb):
        i0 = qb * Q
        kt_lo = max(0, i0 - W + 1) // 128
        kt_hi = (i0 + Q - 1) // 128
        kts = list(range(kt_lo, kt_hi + 1))
        py_a = ps_y.tile([128, Q], F32, tag="yA")
        py_b = ps_y.tile([128, Q], F32, tag="yB")
        for idx, kt in enumerate(kts):
            d = i0 - kt * 128
            pss = ps_s.tile([128, 2, 512], F32, tag="s")
            nc.tensor.matmul(pss[:, 0, 0:Q],
                             kT_[0:64, kt * 128:(kt + 1) * 128],
                             qT[0:64, i0:i0 + Q], start=True, stop=True)
            nc.tensor.matmul(pss[:, 1, 0:Q],
                             kT_[64:128, kt * 128:(kt + 1) * 128],
                             qT[64:128, i0:i0 + Q], start=True, stop=True)
            p_sb = ppool.tile([128, 2 * Q], F32R, tag="p")
            nc.scalar.activation(
                p_sb[:].rearrange("p (r q) -> p r q", r=2),
                pss[:, :, 0:Q], mybir.ActivationFunctionType.Exp)
            midx = _full_mask_idx(d) if is_full else mask_d.index(d)
            if midx is not None:
                mm = m_sb[:, midx, :].rearrange(
                    "p (a q) -> p a q", a=1).broadcast_to([128, 2, Q])
                nc.vector.tensor_mul(
                    p_sb[:].rearrange("p (r q) -> p r q", r=2),
                    p_sb[:].rearrange("p (r q) -> p r q", r=2), mm)
            nc.tensor.matmul(py_a[:], v_sb[:, kt, heads[0], :], p_sb[:, 0:Q],
                             start=(idx == 0), stop=(idx == len(kts) - 1))
            nc.tensor.matmul(py_b[:], v_sb[:, kt, heads[1], :], p_sb[:, Q:2 * Q],
                             start=(idx == 0), stop=(idx == len(kts) - 1))
        # normalize: yT rows = py[0:64] * reciprocal(denominator rows)
        for py, rows in ((py_a, slice(0, 64)), (py_b, slice(64, 128))):
            r_sb = rpool.tile([64, Q], F32, tag="r")
            nc.vector.reciprocal(r_sb[:], py[64:128, :])
            nc.vector.tensor_mul(yT[rows, i0:i0 + Q], py[0:64, :], r_sb[:])

    for f in range(T // QF):
        attn_block(qTf, kTf, QF, WF, mf_sb, MASKF_D, (0, 1), yTf, True, f)
        for rqb in (2 * f, 2 * f + 1):
            attn_block(qTr, kTr, QR, WR, mr_sb, MASKR_D, (2, 3), yTr, False, rqb)
        # c_proj for the 4 T-tiles this region covers
        for tt in range(4 * f, 4 * f + 4):
            tsl = slice(tt * 128, (tt + 1) * 128)
            for nb in range(2):
                nsl = slice(nb * 512, (nb + 1) * 512)
                pso = ps_misc.tile([128, 512], F32, tag="m")
                nc.tensor.matmul(pso[:], yTf[:, tsl], wproj_sb[:, 0, nsl],
                                 start=True, stop=False)
                nc.tensor.matmul(pso[:], yTr[:, tsl], wproj_sb[:, 1, nsl],
                                 start=False, stop=True)
                o_sb = opool.tile([128, 512], F32, tag="osb")
                if nb == 0:
                    nc.scalar.copy(o_sb[:], pso[:])
                else:
                    nc.vector.tensor_copy(o_sb[:], pso[:])
                nc.sync.dma_start(out[tsl, nsl], o_sb[:])


def _build_nc(reps=1):
    nc = bacc.Bacc(trn_type="TRN2", target_bir_lowering=False, debug=False,
                   num_devices=1)

    xT = nc.dram_tensor("xT", [C, T], F32R, kind="ExternalInput").ap()
    wq = nc.dram_tensor("wq", [C, 128], F32R, kind="ExternalInput").ap()
    wk = nc.dram_tensor("wk", [C, 128], F32R, kind="ExternalInput").ap()
    wqr = nc.dram_tensor("wqr", [C, 128], F32R, kind="ExternalInput").ap()
    wkr = nc.dram_tensor("wkr", [C, 128], F32R, kind="ExternalInput").ap()
    wv = nc.dram_tensor("wv", [C, 256], F32R, kind="ExternalInput").ap()
    wproj = nc.dram_tensor("wproj", [256, C], F32R, kind="ExternalInput").ap()
    out = nc.dram_tensor("o", [T, C], F32, kind="ExternalOutput").ap()
    aps = (xT, wq, wk, wqr, wkr, wv, wproj, out)

    with TileContext(nc) as tc:
        with (
            tc.tile_pool(name="wpool", bufs=1) as wpool,
            tc.tile_pool(name="xpool", bufs=10) as xpool,
            tc.tile_pool(name="qk", bufs=1) as qkpool,
            tc.tile_pool(name="ppool", bufs=4) as ppool,
            tc.tile_pool(name="opool", bufs=3) as opool,
            tc.tile_pool(name="rpool", bufs=4) as rpool,
            tc.tile_pool(name="ps_misc", bufs=2, space="PSUM") as ps_misc,
            tc.tile_pool(name="ps_s", bufs=2, space="PSUM") as ps_s,
            tc.tile_pool(name="ps_y", bufs=1, space="PSUM") as ps_y,
        ):
            pools = (wpool, xpool, qkpool, ppool, opool, rpool,
                     ps_misc, ps_s, ps_y)
            for _ in range(reps):
                _emit_body(nc, pools, aps)

    nc.compile()
    return nc


_NC_CACHE = {}


def _get_nc(reps=1):
    if reps not in _NC_CACHE:
        _NC_CACHE[reps] = _build_nc(reps)
    return _NC_CACHE[reps]


def make_in_maps(x, w_qkv_full, w_qk_red, w_v_red, w_proj):
    x = np.asarray(x, np.float32)
    w_qkv_full = np.asarray(w_qkv_full, np.float32)
    w_qk_red = np.asarray(w_qk_red, np.float32)
    w_v_red = np.asarray(w_v_red, np.float32)
    w_proj = np.asarray(w_proj, np.float32)
    sf = np.float32(1.0 / np.sqrt(HDIM))
    sr = np.float32(1.0 / np.sqrt(RDIM))
    in_maps = []
    for c in range(N_CORES):
        b, g = divmod(c, 4)
        hA, hB = 2 * g, 2 * g + 1
        wq = np.concatenate([w_qkv_full[:, 64 * hA:64 * hA + 64],
                             w_qkv_full[:, 64 * hB:64 * hB + 64]], 1) * sf
        wk = np.concatenate([w_qkv_full[:, 512 + 64 * hA:512 + 64 * hA + 64],
                             w_qkv_full[:, 512 + 64 * hB:512 + 64 * hB + 64]], 1)
        wqr = np.zeros((C, 128), np.float32)
        wqr[:, 0:32] = w_qk_red[:, 32 * hA:32 * hA + 32] * sr
        wqr[:, 64:96] = w_qk_red[:, 32 * hB:32 * hB + 32] * sr
        wkr = np.zeros((C, 128), np.float32)
        wkr[:, 0:32] = w_qk_red[:, 256 + 32 * hA:256 + 32 * hA + 32]
        wkr[:, 64:96] = w_qk_red[:, 256 + 32 * hB:256 + 32 * hB + 32]
        wv = np.concatenate([w_qkv_full[:, 1024 + 64 * hA:1024 + 64 * hA + 64],
                             w_qkv_full[:, 1024 + 64 * hB:1024 + 64 * hB + 64],
                             w_v_red[:, 64 * hA:64 * hA + 64],
                             w_v_red[:, 64 * hB:64 * hB + 64]], 1)
        wp = np.concatenate([w_proj[64 * hA:64 * hA + 64, :],
                             w_proj[64 * hB:64 * hB + 64, :],
                             w_proj[512 + 64 * hA:512 + 64 * hA + 64, :],
                             w_proj[512 + 64 * hB:512 + 64 * hB + 64, :]], 0)
        in_maps.append({
            "xT": np.ascontiguousarray(x[b].T),
            "wq": np.ascontiguousarray(wq), "wk": np.ascontiguousarray(wk),
            "wqr": wqr, "wkr": wkr, "wv": np.ascontiguousarray(wv),
            "wproj": np.ascontiguousarray(wp),
        })
    return in_maps


def kernel(x, w_qkv_full, w_qk_red, w_v_red, w_proj):
    nc = _get_nc()
    in_maps = make_in_maps(x, w_qkv_full, w_qk_red, w_v_red, w_proj)
    r = bass_utils.run_bass_kernel_spmd(nc, in_maps,
                                        core_ids=list(range(N_CORES)),
                                        trace=False)
    outs = [r.results[c]["o"] for c in range(N_CORES)]
    y = np.zeros((B, T, C), np.float32)
    for b in range(B):
        y[b] = outs[4 * b] + outs[4 * b + 1] + outs[4 * b + 2] + outs[4 * b + 3]
    return y
